# revision 38
# baseline (speedup 1.0000x reference)
"""Trainium2 Bass kernel for a 2-layer ConvLSTM block (B=4,T=8,64x64,C=F=32).

Sharding: 8 cores = batch(4) x H-halves(2). Each core computes 40 of 64 rows
(32 owned + 8 redundant ghost rows) so NO cross-core communication is needed.
Bottom-half cores get vertically flipped inputs and kh-flipped weights so all
8 cores run one SPMD program.

Device layout (v2, gate-major / fold-free):
  - px space: 66-wide rows (64 valid + 2 junk edge cols), 40 rows = 2640 px.
  - 3x3 convs as matmuls, weights stationary [K<=102, M=128=(gate,chan)].
    im2col buffers hold 3 dx-shifted copies in partition blocks; dy taps
    index the row window. 6 matmuls (3 x-taps + 3 h-taps) accumulate one
    7-row (462 col) f32 PSUM tile; 6 tiles cover a step.
  - the hard-sigmoid affine (0.2 z + 0.5) is folded into the i/f/o gate
    weights+bias on the HOST, so the sigmoid drain is a pure clamp
    (z max 0) min 1 on DVE; the c-gate drain is Tanh on ACT with a
    partition-base shift 96->0 (single-tensor engine ops may shift bases;
    tensor-tensor ops must be same-base, which drives the tile layout).
  - gate math on 32-partition tiles: t1 = sig_i * tanh_g; c = c*sig_f + t1
    (sig_f first copied base 32->0); th = tanh(c) written at base 64 so
    h = sig_o * th is a legal same-base op on partitions 64:96.
  - h tiles [*, 2642]: 1-col slack each end, junk cols always 0. The
    "unfold" of h into the recurrent im2col buffer is 3 plain dense DMAs:
    dx-shifted copies are flat shifts by -1/0/+1 elements in px space (the
    zero junk cols provide exactly the conv halo zeros).
  - BN2 is folded into layer-2 input weights (scale) and bias (shift), with
    SAME-padding boundary effects restored by 5 constant indicator K-rows
    (left/right/top edges + 2 corners) in the shared im2col buffer: layer 2
    reads layer 1's recurrent buffer directly, no separate BN2 pass/unfold.
  - residual h2+h1 on-chip; h1 tiles persist in SBUF (no DRAM spill).
Host does: BN1 fold into x, im2col staging of x, weight prep (gate reorder
[i,f,c,o]->[i,f,o,c], hsig/BN2/boundary folds, bias row, vertical flip),
out decode.
"""
import numpy as np

import concourse.bass as bass
import concourse.tile as tile
from concourse import mybir
from concourse.bass_utils import run_bass_kernel_spmd

F32 = mybir.dt.float32
BF16 = mybir.dt.bfloat16
AF = mybir.ActivationFunctionType
ALU = mybir.AluOpType

B, T, H, W, C, F = 4, 8, 64, 64, 32, 32
L = 2
BN_EPS = 1e-3
R = 40          # compute rows per core
RD = 41         # data rows of x each core needs (R + 1 halo)
RR = 42         # padded row slots in im2col buffers
WW = 66         # padded row width (64 valid + 2 junk cols)
NPX = R * WW    # 2640 px per step (incl. junk cols)
KX2 = 102       # layer-2 x-conv K rows: 96 + ones + 5 boundary indicators
NCORES = 8
# psum tiles: 7-row chunks (462 f32 <= one 2KB bank per partition)
TILES = [(0, 7), (7, 7), (14, 7), (21, 7), (28, 7), (35, 5)]

_PROG = {}


def _split_excess_waits(nc, max_waits=1):
    """This walrus rejects >1 sync-wait per instruction on some engines; move
    excess waits onto NoOps inserted just before, on the same engine."""
    for fn in nc.m.functions:
        for bb in fn.blocks:
            new_insts = []
            for inst in bb.instructions:
                si = inst.sync_info
                waits = list(si.on_wait) if si and si.on_wait else []
                if len(waits) > max_waits:
                    k = 0
                    while len(waits) - k > max_waits:
                        chunk = waits[k:k + max_waits]
                        k += max_waits
                        new_insts.append(mybir.InstNoOp(
                            name=f"waitsplit_{inst.name}_{k}",
                            engine=inst.engine,
                            sync_info=mybir.SyncInfo(on_wait=list(chunk),
                                                     on_update=[]),
                        ))
                    inst.sync_info = mybir.SyncInfo(
                        on_wait=list(waits[k:]), on_update=list(si.on_update))
                new_insts.append(inst)
            bb.instructions = new_insts


def _build_program():
    nc = bass.Bass("TRN2", target_bir_lowering=False, debug=False)

    xim_d = nc.dram_tensor("xim", [97, T, RR, WW], BF16, kind="ExternalInput").ap()
    w1_d = nc.dram_tensor("w1", [3, 97, 128], BF16, kind="ExternalInput").ap()
    u1_d = nc.dram_tensor("u1", [3, 96, 128], BF16, kind="ExternalInput").ap()
    w2_d = nc.dram_tensor("w2", [3, KX2, 128], BF16, kind="ExternalInput").ap()
    u2_d = nc.dram_tensor("u2", [3, 96, 128], BF16, kind="ExternalInput").ap()
    ind_d = nc.dram_tensor("ind", [6, RR, WW], BF16, kind="ExternalInput").ap()
    out_d = nc.dram_tensor("out", [T, 32, NPX], BF16, kind="ExternalOutput").ap()

    with tile.TileContext(nc) as tc:
        with tc.tile_pool(name="const", bufs=1) as constp, \
             tc.tile_pool(name="ysp", bufs=1) as ysp, \
             tc.tile_pool(name="ximp", bufs=3) as ximp, \
             tc.tile_pool(name="sgp", bufs=4) as sgp, \
             tc.tile_pool(name="tmp", bufs=3) as tmpp, \
             tc.tile_pool(name="outp", bufs=2) as outp, \
             tc.tile_pool(name="ps", bufs=6, space="PSUM") as psp:

            # ---- weights ----
            wt = {}
            for nm, src, kk in (("w1", w1_d, 97), ("u1", u1_d, 96),
                                ("w2", w2_d, KX2), ("u2", u2_d, 96)):
                for dy in range(3):
                    t_ = constp.tile([kk, 128], BF16, tag=f"{nm}{dy}",
                                     name=f"{nm}{dy}")
                    nc.sync.dma_start(t_[:], src[dy])
                    wt[(nm, dy)] = t_

            # ---- persistent state ----
            # ys1h [102, RR, WW]: rows 0:96 = 3 dx-shifted h copies; row 96 =
            # ones (bias); rows 97:102 = constant boundary indicators for the
            # BN2 fold (left col, right col, top row, TL corner, TR corner).
            # Serves BOTH layer-1 recurrence (rows 0:96) and layer-2 input
            # conv (all rows). ys2h [96]: layer-2 recurrence.
            ys = {}
            for j, (nm, pp) in enumerate(x for x in (("ys1h", KX2),
                                                     ("ys2h", 96))):
                for i in range(2):
                    t_ = ysp.tile([pp, RR, WW], BF16, tag=f"{nm}{i}",
                                  name=f"{nm}{i}")
                    (nc.vector if i == 0 else nc.gpsimd).memset(t_[0:96], 0.0)
                    if pp == KX2:
                        nc.sync.dma_start(t_[96:102], ind_d)
                    ys[(nm, i)] = t_
            c_st = {}
            for l in (1, 2):
                t_ = ysp.tile([32, NPX], BF16, tag=f"c{l}", name=f"c{l}")
                nc.vector.memset(t_[:], 0.0)
                c_st[l] = t_
            # h tiles live at partition base 64 (slice of a 96-row tile) so
            # h = sig_o * th is same-base; [.., NPX+2] slack + zero junk cols.
            ht = {}
            for l in (1, 2):
                for i in range(2):
                    t_ = ysp.tile([96, NPX + 2], BF16, tag=f"h{l}_{i}",
                                  name=f"h{l}_{i}")
                    (nc.vector if i == 0 else nc.gpsimd).memset(t_[64:96], 0.0)
                    ht[(l, i)] = t_

            ximt_tiles = {}

            def prefetch(t):
                if t < T and t not in ximt_tiles:
                    t_ = ximp.tile([97, RR, WW], BF16, name="ximt")
                    nc.sync.dma_start(t_[:], xim_d[:, t])
                    ximt_tiles[t] = t_

            # Adaptive ghost-row shrink: layer-l step-t only needs this many
            # valid rows (validity shrinks 1 row per remaining conv).
            def crows(l, t):
                return (40 - t) if l == 1 else (39 - t)

            def halves(l, t):
                """(r0, r1, tiles) chunks covering rows [0, crows)."""
                c = crows(l, t)
                tl = [(7 * k, min(7, c - 7 * k)) for k in range((c + 6) // 7)]
                return ((0, 21, tl[0:3]), (21, c, tl[3:]))

            def conv_half(l, t, half, sig, zg):
                if l == 1:
                    ximt = ximt_tiles[t]
                    kx = 97
                else:
                    ximt = ys[("ys1h", t % 2)]
                    kx = KX2
                ysh = ys[(f"ys{l}h", (t + 1) % 2)]
                wx = [wt[(f"w{l}", dy)] for dy in range(3)]
                uh = [wt[(f"u{l}", dy)] for dy in range(3)]
                for r0, nr in halves(l, t)[half][2]:
                    zp = psp.tile([128, 66 * nr], F32, name="zp")
                    for i, dy in enumerate(range(3)):
                        nc.tensor.matmul(zp[:], wx[dy][0:kx],
                                         ximt[0:kx, r0 + dy:r0 + dy + nr, :],
                                         start=(i == 0), stop=False)
                    for i, dy in enumerate(range(3)):
                        nc.tensor.matmul(zp[:], uh[dy][:],
                                         ysh[0:96, r0 + dy:r0 + dy + nr, :],
                                         start=False, stop=(i == 2))
                    csl = slice(66 * r0, 66 * (r0 + nr))
                    # hsig = clamp (affine folded into weights); tanh drains
                    # shift the c-gate block from base 96 to 0.
                    nc.vector.tensor_scalar(sig[:, csl], zp[0:96, :],
                                            0.0, 1.0, ALU.max, ALU.min)
                    nc.scalar.activation(zg[:, csl], zp[96:128, :], AF.Tanh)

            def gates_half(l, t, half, sig, zg, tmps):
                ct = c_st[l]
                h = ht[(l, t % 2)]
                t1, fsh, th = tmps
                r0, r1, _ = halves(l, t)[half]
                sl = slice(66 * r0, 66 * r1)
                nc.scalar.copy(fsh[:, sl], sig[32:64, sl])         # f -> base0
                nc.gpsimd.tensor_mul(t1[:, sl], sig[0:32, sl],
                                     zg[:, sl])                    # i * g
                nc.vector.tensor_mul(ct[:, sl], ct[:, sl],
                                     fsh[:, sl])                   # c *= f
                nc.vector.tensor_add(ct[:, sl], ct[:, sl], t1[:, sl])
                qm = (r0 + r1) // 2
                for q0, q1 in ((r0, qm), (qm, r1)):
                    qsl = slice(66 * q0, 66 * q1)
                    nc.scalar.activation(th[64:96, qsl], ct[:, qsl],
                                         AF.Tanh)                  # -> base64
                # h = sig_o * th on partitions 64:96, skip junk cols
                ov = sig[64:96, sl].rearrange("c (r w) -> c r w",
                                              w=WW)[:, :, 1:65]
                tv = th[64:96, sl].rearrange("c (r w) -> c r w",
                                             w=WW)[:, :, 1:65]
                hv = h[64:96, 2 + 66 * r0:2 + 66 * r1].rearrange(
                    "c (r w) -> c r w", w=WW)[:, :, 0:64]
                nc.vector.tensor_mul(hv, ov, tv)

            def gates_tail(l, t):
                h = ht[(l, t % 2)]
                if l == 1 or t < T - 1:
                    dst = ys[(f"ys{l}h", t % 2)]
                    npx = 66 * crows(l, t)
                    # all three on Pool's queue (SWDGE): its only other work
                    # precedes these naturally, so the h-wait here never
                    # blocks foreign instructions (SP/ACT queues would stall
                    # their next-slot work behind it)
                    for eng, dx in ((nc.sync, 0), (nc.scalar, 1),
                                    (nc.gpsimd, 2)):
                        s = dx - 1
                        eng.dma_start(
                            dst[32 * dx:32 * dx + 32].rearrange(
                                "c r w -> c (r w)")[:, WW:WW + npx],
                            h[64:96, 1 + s:1 + s + npx])
                if l == 2:
                    # only the 32 owned rows matter downstream
                    res = outp.tile([96, 66 * 32], BF16, name="res")
                    h1 = ht[(1, t % 2)]
                    nc.gpsimd.tensor_add(res[64:96, :],
                                         h[64:96, 1:1 + 66 * 32],
                                         h1[64:96, 1:1 + 66 * 32])
                    nc.sync.dma_start(out_d[t, :, 0:66 * 32], res[64:96, :])

            # Software pipelining with half-granular interleave: each engine's
            # in-order queue sees work in data-readiness order, so the next
            # slot's drains never queue behind a whole gate tail.
            def alloc_sgz():
                sig = sgp.tile([96, NPX], BF16, tag="sig", name="sig")
                zg = sgp.tile([32, NPX], BF16, tag="zg", name="zg")
                return sig, zg

            def alloc_tmps():
                t1 = tmpp.tile([32, NPX], BF16, tag="t1", name="t1")
                fsh = tmpp.tile([32, NPX], BF16, tag="fsh", name="fsh")
                th = tmpp.tile([96, NPX], BF16, tag="th", name="th")
                return t1, fsh, th

            prefetch(0)
            prefetch(1)
            for s in range(T + 1):
                prefetch(s + 2)
                do1 = s < T
                do2 = s >= 1
                if do1:
                    sz1 = alloc_sgz()
                    tm1 = alloc_tmps()
                    conv_half(1, s, 0, *sz1)
                    conv_half(1, s, 1, *sz1)
                    gates_half(1, s, 0, *sz1, tm1)
                if do2:
                    sz2 = alloc_sgz()
                    tm2 = alloc_tmps()
                    conv_half(2, s - 1, 0, *sz2)
                if do1:
                    gates_half(1, s, 1, *sz1, tm1)
                    gates_tail(1, s)
                if do2:
                    conv_half(2, s - 1, 1, *sz2)
                    gates_half(2, s - 1, 0, *sz2, tm2)
                    gates_half(2, s - 1, 1, *sz2, tm2)
                    gates_tail(2, s - 1)

    _split_excess_waits(nc)
    return nc


def _host_prep(x, bn_gamma, bn_beta, bn_mean, bn_var, kernels, rec_kernels,
               biases):
    """Build the 8 per-core input maps."""
    import ml_dtypes
    # gate reorder [i,f,c,o] -> [i,f,o,c]
    perm = np.concatenate([np.arange(0, 64), np.arange(96, 128),
                           np.arange(64, 96)])
    s1 = bn_gamma[0] / np.sqrt(bn_var[0] + BN_EPS)
    t1 = bn_beta[0] - bn_mean[0] * s1
    s2 = bn_gamma[1] / np.sqrt(bn_var[1] + BN_EPS)
    t2 = bn_beta[1] - bn_mean[1] * s2
    y1 = x * s1 + t1                                  # BN1 on host

    def hsig_fold(out):
        out[:, :, 0:96] *= 0.2
        out[1, 96, 0:96] += 0.5
        return out

    def wmat1(wk, bias_vec, flip):
        """layer-1 x-conv: [3,3,C,4F] -> per-dy lhsT [97,128], bias row 96."""
        wk = np.asarray(wk)[::-1] if flip else np.asarray(wk)
        out = np.zeros((3, 97, 128), np.float32)
        for dy in range(3):
            out[dy, :96] = wk[dy].reshape(96, 128)[:, perm]
        out[1, 96] = bias_vec[perm]
        return hsig_fold(out)

    def wmat2(wk, bias_vec, flip):
        """layer-2 x-conv with BN2 folded: scale into weights, shift into
        bias, boundary indicator columns (rows 97:102) restore SAME-padding
        (reference pads with literal zeros AFTER BN2)."""
        wk = np.asarray(wk).astype(np.float64)
        bias = bias_vec.astype(np.float64) + np.einsum("yxcm,c->m", wk, t2)
        wkf = wk[::-1] if flip else wk
        out = np.zeros((3, KX2, 128), np.float32)
        for dy in range(3):
            out[dy, :96] = (wkf[dy] * s2[:, None]).reshape(96, 128)[:, perm]
        out[1, 96] = bias[perm]
        out[1, 97] = -np.einsum("ycm,c->m", wkf[:, 0], t2)[perm]   # left col
        out[1, 98] = -np.einsum("ycm,c->m", wkf[:, 2], t2)[perm]   # right col
        out[1, 99] = -np.einsum("xcm,c->m", wkf[0, :], t2)[perm]   # top row
        out[1, 100] = np.einsum("cm,c->m", wkf[0, 0], t2)[perm]    # TL corner
        out[1, 101] = np.einsum("cm,c->m", wkf[0, 2], t2)[perm]    # TR corner
        return hsig_fold(out)

    def umat(wk, flip):
        wk = np.asarray(wk)[::-1] if flip else np.asarray(wk)
        out = np.zeros((3, 96, 128), np.float32)
        for dy in range(3):
            out[dy] = wk[dy].reshape(96, 128)[:, perm]
        out[:, :, 0:96] *= 0.2
        return out

    in_maps = []
    for core in range(NCORES):
        b, half = core // 2, core % 2
        yb = y1[b] if half == 0 else y1[b, :, ::-1]
        yb = np.ascontiguousarray(yb[:, :RD])         # [T, 41, 64, 32]
        xim = np.zeros((97, T, RR, WW), np.float32)
        yt = yb.transpose(3, 0, 1, 2)                 # [32, T, 41, 64]
        for dx in range(3):
            xim[32 * dx:32 * dx + 32, :, 1:RR, 2 - dx:WW - dx] = yt
        xim[96] = 1.0
        ind = np.zeros((6, RR, WW), np.float32)
        ind[0] = 1.0            # ones / bias row
        ind[1, :, 1] = 1.0      # left col indicator (w=0)
        ind[2, :, 64] = 1.0     # right col (w=63)
        ind[3, 1, :] = 1.0      # top row (r=0)
        ind[4, 1, 1] = 1.0      # TL corner
        ind[5, 1, 64] = 1.0     # TR corner
        flip = half == 1
        in_maps.append({
            "xim": xim.astype(ml_dtypes.bfloat16),
            "w1": wmat1(kernels[0], biases[0], flip).astype(ml_dtypes.bfloat16),
            "u1": umat(rec_kernels[0], flip).astype(ml_dtypes.bfloat16),
            "w2": wmat2(kernels[1], biases[1], flip).astype(ml_dtypes.bfloat16),
            "u2": umat(rec_kernels[1], flip).astype(ml_dtypes.bfloat16),
            "ind": ind.astype(ml_dtypes.bfloat16),
        })
    return in_maps


def _decode(results):
    """Per-core out [T, 32, 2640] -> full [B,T,H,W,C]."""
    out = np.zeros((B, T, H, W, C), np.float32)
    for core in range(NCORES):
        o = results[core]["out"].astype(np.float32)   # [T, 32, 2640]
        o = o.reshape(T, 32, R, WW)[:, :, 0:32, 1:65]  # owned rows, valid w
        o = o.transpose(0, 2, 3, 1)                   # [T, 32, 64, 32]
        b, half = core // 2, core % 2
        if half == 0:
            out[b, :, 0:32] = o
        else:
            out[b, :, 32:64] = o[:, ::-1]
    return out


def kernel(**inputs):
    if "nc" not in _PROG:
        _PROG["nc"] = _build_program()
    in_maps = _host_prep(**inputs)
    res = run_bass_kernel_spmd(_PROG["nc"], in_maps, list(range(NCORES)))
    return _decode(res.results)


# revision 39
# speedup vs baseline: 1.1038x; 1.1038x over previous
"""Trainium2 Bass kernel for a 2-layer ConvLSTM block (B=4,T=8,64x64,C=F=32).

Sharding: 8 cores = batch(4) x H-halves(2). Each core computes 40 of 64 rows
(32 owned + 8 redundant ghost rows) so NO cross-core communication is needed.
Bottom-half cores get vertically flipped inputs and kh-flipped weights so all
8 cores run one SPMD program.

Device layout (v2, gate-major / fold-free):
  - px space: 66-wide rows (64 valid + 2 junk edge cols), 40 rows = 2640 px.
  - 3x3 convs as matmuls, weights stationary [K<=102, M=128=(gate,chan)].
    im2col buffers hold 3 dx-shifted copies in partition blocks; dy taps
    index the row window. 6 matmuls (3 x-taps + 3 h-taps) accumulate one
    7-row (462 col) f32 PSUM tile; 6 tiles cover a step.
  - the hard-sigmoid affine (0.2 z + 0.5) is folded into the i/f/o gate
    weights+bias on the HOST, so the sigmoid drain is a pure clamp
    (z max 0) min 1 on DVE; the c-gate drain is Tanh on ACT with a
    partition-base shift 96->0 (single-tensor engine ops may shift bases;
    tensor-tensor ops must be same-base, which drives the tile layout).
  - gate math on 32-partition tiles: t1 = sig_i * tanh_g; c = c*sig_f + t1
    (sig_f first copied base 32->0); th = tanh(c) written at base 64 so
    h = sig_o * th is a legal same-base op on partitions 64:96.
  - h tiles [*, 2642]: 1-col slack each end, junk cols always 0. The
    "unfold" of h into the recurrent im2col buffer is 3 plain dense DMAs:
    dx-shifted copies are flat shifts by -1/0/+1 elements in px space (the
    zero junk cols provide exactly the conv halo zeros).
  - BN2 is folded into layer-2 input weights (scale) and bias (shift), with
    SAME-padding boundary effects restored by 5 constant indicator K-rows
    (left/right/top edges + 2 corners) in the shared im2col buffer: layer 2
    reads layer 1's recurrent buffer directly, no separate BN2 pass/unfold.
  - residual h2+h1 on-chip; h1 tiles persist in SBUF (no DRAM spill).
Host does: BN1 fold into x, im2col staging of x, weight prep (gate reorder
[i,f,c,o]->[i,f,o,c], hsig/BN2/boundary folds, bias row, vertical flip),
out decode.
"""
import numpy as np

import concourse.bass as bass
import concourse.tile as tile
from concourse import mybir
from concourse.bass_utils import run_bass_kernel_spmd

F32 = mybir.dt.float32
BF16 = mybir.dt.bfloat16
AF = mybir.ActivationFunctionType
ALU = mybir.AluOpType

B, T, H, W, C, F = 4, 8, 64, 64, 32, 32
L = 2
BN_EPS = 1e-3
R = 40          # compute rows per core
RD = 41         # data rows of x each core needs (R + 1 halo)
RR = 42         # padded row slots in im2col buffers
WW = 66         # padded row width (64 valid + 2 junk cols)
NPX = R * WW    # 2640 px per step (incl. junk cols)
KX2 = 102       # layer-2 x-conv K rows: 96 + ones + 5 boundary indicators
NCORES = 8
# psum tiles: 7-row chunks (462 f32 <= one 2KB bank per partition)
TILES = [(0, 7), (7, 7), (14, 7), (21, 7), (28, 7), (35, 5)]

_PROG = {}


def _split_excess_waits(nc, max_waits=1):
    """This walrus rejects >1 sync-wait per instruction on some engines; move
    excess waits onto NoOps inserted just before, on the same engine."""
    for fn in nc.m.functions:
        for bb in fn.blocks:
            new_insts = []
            for inst in bb.instructions:
                si = inst.sync_info
                waits = list(si.on_wait) if si and si.on_wait else []
                if len(waits) > max_waits:
                    k = 0
                    while len(waits) - k > max_waits:
                        chunk = waits[k:k + max_waits]
                        k += max_waits
                        new_insts.append(mybir.InstNoOp(
                            name=f"waitsplit_{inst.name}_{k}",
                            engine=inst.engine,
                            sync_info=mybir.SyncInfo(on_wait=list(chunk),
                                                     on_update=[]),
                        ))
                    inst.sync_info = mybir.SyncInfo(
                        on_wait=list(waits[k:]), on_update=list(si.on_update))
                new_insts.append(inst)
            bb.instructions = new_insts


def _build_program():
    nc = bass.Bass("TRN2", target_bir_lowering=False, debug=False)

    xim_d = nc.dram_tensor("xim", [97, T, RR, WW], BF16, kind="ExternalInput").ap()
    w1_d = nc.dram_tensor("w1", [3, 97, 128], BF16, kind="ExternalInput").ap()
    u1_d = nc.dram_tensor("u1", [3, 96, 128], BF16, kind="ExternalInput").ap()
    w2_d = nc.dram_tensor("w2", [3, KX2, 128], BF16, kind="ExternalInput").ap()
    u2_d = nc.dram_tensor("u2", [3, 96, 128], BF16, kind="ExternalInput").ap()
    ind_d = nc.dram_tensor("ind", [6, RR, WW], BF16, kind="ExternalInput").ap()
    out_d = nc.dram_tensor("out", [T, 32, NPX], BF16, kind="ExternalOutput").ap()

    with tile.TileContext(nc) as tc:
        with tc.tile_pool(name="const", bufs=1) as constp, \
             tc.tile_pool(name="ysp", bufs=1) as ysp, \
             tc.tile_pool(name="ximp", bufs=3) as ximp, \
             tc.tile_pool(name="sgp", bufs=4) as sgp, \
             tc.tile_pool(name="tmp", bufs=3) as tmpp, \
             tc.tile_pool(name="outp", bufs=2) as outp, \
             tc.tile_pool(name="ps", bufs=6, space="PSUM") as psp:

            # ---- weights ----
            wt = {}
            for nm, src, kk in (("w1", w1_d, 97), ("u1", u1_d, 96),
                                ("w2", w2_d, KX2), ("u2", u2_d, 96)):
                for dy in range(3):
                    t_ = constp.tile([kk, 128], BF16, tag=f"{nm}{dy}",
                                     name=f"{nm}{dy}")
                    nc.sync.dma_start(t_[:], src[dy])
                    wt[(nm, dy)] = t_

            # ---- persistent state ----
            # ys1h [102, RR, WW]: rows 0:96 = 3 dx-shifted h copies; row 96 =
            # ones (bias); rows 97:102 = constant boundary indicators for the
            # BN2 fold (left col, right col, top row, TL corner, TR corner).
            # Serves BOTH layer-1 recurrence (rows 0:96) and layer-2 input
            # conv (all rows). ys2h [96]: layer-2 recurrence.
            ys = {}
            for j, (nm, pp) in enumerate(x for x in (("ys1h", KX2),
                                                     ("ys2h", 96))):
                for i in range(2):
                    t_ = ysp.tile([pp, RR, WW], BF16, tag=f"{nm}{i}",
                                  name=f"{nm}{i}")
                    (nc.vector if i == 0 else nc.gpsimd).memset(t_[0:96], 0.0)
                    if pp == KX2:
                        nc.sync.dma_start(t_[96:102], ind_d)
                    ys[(nm, i)] = t_
            c_st = {}
            for l in (1, 2):
                t_ = ysp.tile([32, NPX], BF16, tag=f"c{l}", name=f"c{l}")
                nc.vector.memset(t_[:], 0.0)
                c_st[l] = t_
            # h tiles live at partition base 64 (slice of a 96-row tile) so
            # h = sig_o * th is same-base; [.., NPX+2] slack + zero junk cols.
            ht = {}
            for l in (1, 2):
                for i in range(2):
                    t_ = ysp.tile([96, NPX + 2], BF16, tag=f"h{l}_{i}",
                                  name=f"h{l}_{i}")
                    (nc.vector if i == 0 else nc.gpsimd).memset(t_[64:96], 0.0)
                    ht[(l, i)] = t_

            ximt_tiles = {}

            def prefetch(t):
                if t < T and t not in ximt_tiles:
                    t_ = ximp.tile([97, RR, WW], BF16, name="ximt")
                    nc.sync.dma_start(t_[:], xim_d[:, t])
                    ximt_tiles[t] = t_

            # Adaptive ghost-row shrink: layer-l step-t only needs this many
            # valid rows (validity shrinks 1 row per remaining conv).
            def crows(l, t):
                return (40 - t) if l == 1 else (39 - t)

            def halves(l, t):
                """(r0, r1, tiles) chunks covering rows [0, crows)."""
                c = crows(l, t)
                tl = [(7 * k, min(7, c - 7 * k)) for k in range((c + 6) // 7)]
                return ((0, 21, tl[0:3]), (21, c, tl[3:]))

            def conv_half(l, t, half, sig, zg):
                if l == 1:
                    ximt = ximt_tiles[t]
                    kx = 97
                else:
                    ximt = ys[("ys1h", t % 2)]
                    kx = KX2
                ysh = ys[(f"ys{l}h", (t + 1) % 2)]
                wx = [wt[(f"w{l}", dy)] for dy in range(3)]
                uh = [wt[(f"u{l}", dy)] for dy in range(3)]
                for r0, nr in halves(l, t)[half][2]:
                    zp = psp.tile([128, 66 * nr], F32, name="zp")
                    for i, dy in enumerate(range(3)):
                        nc.tensor.matmul(zp[:], wx[dy][0:kx],
                                         ximt[0:kx, r0 + dy:r0 + dy + nr, :],
                                         start=(i == 0), stop=False)
                    for i, dy in enumerate(range(3)):
                        nc.tensor.matmul(zp[:], uh[dy][:],
                                         ysh[0:96, r0 + dy:r0 + dy + nr, :],
                                         start=False, stop=(i == 2))
                    csl = slice(66 * r0, 66 * (r0 + nr))
                    # hsig = clamp (affine folded into weights); tanh drains
                    # shift the c-gate block from base 96 to 0.
                    nc.vector.tensor_scalar(sig[:, csl], zp[0:96, :],
                                            0.0, 1.0, ALU.max, ALU.min)
                    nc.scalar.activation(zg[:, csl], zp[96:128, :], AF.Tanh)

            def gates_half(l, t, half, sig, zg, tmps):
                ct = c_st[l]
                h = ht[(l, t % 2)]
                t1, fsh, th = tmps
                r0, r1, _ = halves(l, t)[half]
                sl = slice(66 * r0, 66 * r1)
                nc.vector.tensor_scalar(fsh[:, sl], sig[32:64, sl],
                                        0.0, None, ALU.max)        # f -> base0
                nc.gpsimd.tensor_mul(t1[:, sl], sig[0:32, sl],
                                     zg[:, sl])                    # i * g
                nc.vector.tensor_mul(ct[:, sl], ct[:, sl],
                                     fsh[:, sl])                   # c *= f
                nc.vector.tensor_add(ct[:, sl], ct[:, sl], t1[:, sl])
                qm = (r0 + r1) // 2
                for q0, q1 in ((r0, qm), (qm, r1)):
                    qsl = slice(66 * q0, 66 * q1)
                    nc.scalar.activation(th[64:96, qsl], ct[:, qsl],
                                         AF.Tanh)                  # -> base64
                # h = sig_o * th on partitions 64:96, skip junk cols
                ov = sig[64:96, sl].rearrange("c (r w) -> c r w",
                                              w=WW)[:, :, 1:65]
                tv = th[64:96, sl].rearrange("c (r w) -> c r w",
                                             w=WW)[:, :, 1:65]
                hv = h[64:96, 2 + 66 * r0:2 + 66 * r1].rearrange(
                    "c (r w) -> c r w", w=WW)[:, :, 0:64]
                nc.vector.tensor_mul(hv, ov, tv)

            def gates_tail(l, t):
                h = ht[(l, t % 2)]
                if l == 1 or t < T - 1:
                    dst = ys[(f"ys{l}h", t % 2)]
                    npx = 66 * crows(l, t)
                    # all three on Pool's queue (SWDGE): its only other work
                    # precedes these naturally, so the h-wait here never
                    # blocks foreign instructions (SP/ACT queues would stall
                    # their next-slot work behind it)
                    for eng, dx in ((nc.sync, 0), (nc.scalar, 1),
                                    (nc.gpsimd, 2)):
                        s = dx - 1
                        eng.dma_start(
                            dst[32 * dx:32 * dx + 32].rearrange(
                                "c r w -> c (r w)")[:, WW:WW + npx],
                            h[64:96, 1 + s:1 + s + npx])
                if l == 2:
                    # only the 32 owned rows matter downstream
                    res = outp.tile([96, 66 * 32], BF16, name="res")
                    h1 = ht[(1, t % 2)]
                    nc.gpsimd.tensor_add(res[64:96, :],
                                         h[64:96, 1:1 + 66 * 32],
                                         h1[64:96, 1:1 + 66 * 32])
                    nc.sync.dma_start(out_d[t, :, 0:66 * 32], res[64:96, :])

            # Software pipelining with half-granular interleave: each engine's
            # in-order queue sees work in data-readiness order, so the next
            # slot's drains never queue behind a whole gate tail.
            def alloc_sgz():
                sig = sgp.tile([96, NPX], BF16, tag="sig", name="sig")
                zg = sgp.tile([32, NPX], BF16, tag="zg", name="zg")
                return sig, zg

            def alloc_tmps():
                t1 = tmpp.tile([32, NPX], BF16, tag="t1", name="t1")
                fsh = tmpp.tile([32, NPX], BF16, tag="fsh", name="fsh")
                th = tmpp.tile([96, NPX], BF16, tag="th", name="th")
                return t1, fsh, th

            prefetch(0)
            prefetch(1)
            for s in range(T + 1):
                prefetch(s + 2)
                do1 = s < T
                do2 = s >= 1
                if do1:
                    sz1 = alloc_sgz()
                    tm1 = alloc_tmps()
                    conv_half(1, s, 0, *sz1)
                    conv_half(1, s, 1, *sz1)
                    gates_half(1, s, 0, *sz1, tm1)
                if do2:
                    sz2 = alloc_sgz()
                    tm2 = alloc_tmps()
                    conv_half(2, s - 1, 0, *sz2)
                if do1:
                    gates_half(1, s, 1, *sz1, tm1)
                    gates_tail(1, s)
                if do2:
                    conv_half(2, s - 1, 1, *sz2)
                    gates_half(2, s - 1, 0, *sz2, tm2)
                    gates_half(2, s - 1, 1, *sz2, tm2)
                    gates_tail(2, s - 1)

    _split_excess_waits(nc)
    return nc


def _host_prep(x, bn_gamma, bn_beta, bn_mean, bn_var, kernels, rec_kernels,
               biases):
    """Build the 8 per-core input maps."""
    import ml_dtypes
    # gate reorder [i,f,c,o] -> [i,f,o,c]
    perm = np.concatenate([np.arange(0, 64), np.arange(96, 128),
                           np.arange(64, 96)])
    s1 = bn_gamma[0] / np.sqrt(bn_var[0] + BN_EPS)
    t1 = bn_beta[0] - bn_mean[0] * s1
    s2 = bn_gamma[1] / np.sqrt(bn_var[1] + BN_EPS)
    t2 = bn_beta[1] - bn_mean[1] * s2
    y1 = x * s1 + t1                                  # BN1 on host

    def hsig_fold(out):
        out[:, :, 0:96] *= 0.2
        out[1, 96, 0:96] += 0.5
        return out

    def wmat1(wk, bias_vec, flip):
        """layer-1 x-conv: [3,3,C,4F] -> per-dy lhsT [97,128], bias row 96."""
        wk = np.asarray(wk)[::-1] if flip else np.asarray(wk)
        out = np.zeros((3, 97, 128), np.float32)
        for dy in range(3):
            out[dy, :96] = wk[dy].reshape(96, 128)[:, perm]
        out[1, 96] = bias_vec[perm]
        return hsig_fold(out)

    def wmat2(wk, bias_vec, flip):
        """layer-2 x-conv with BN2 folded: scale into weights, shift into
        bias, boundary indicator columns (rows 97:102) restore SAME-padding
        (reference pads with literal zeros AFTER BN2)."""
        wk = np.asarray(wk).astype(np.float64)
        bias = bias_vec.astype(np.float64) + np.einsum("yxcm,c->m", wk, t2)
        wkf = wk[::-1] if flip else wk
        out = np.zeros((3, KX2, 128), np.float32)
        for dy in range(3):
            out[dy, :96] = (wkf[dy] * s2[:, None]).reshape(96, 128)[:, perm]
        out[1, 96] = bias[perm]
        out[1, 97] = -np.einsum("ycm,c->m", wkf[:, 0], t2)[perm]   # left col
        out[1, 98] = -np.einsum("ycm,c->m", wkf[:, 2], t2)[perm]   # right col
        out[1, 99] = -np.einsum("xcm,c->m", wkf[0, :], t2)[perm]   # top row
        out[1, 100] = np.einsum("cm,c->m", wkf[0, 0], t2)[perm]    # TL corner
        out[1, 101] = np.einsum("cm,c->m", wkf[0, 2], t2)[perm]    # TR corner
        return hsig_fold(out)

    def umat(wk, flip):
        wk = np.asarray(wk)[::-1] if flip else np.asarray(wk)
        out = np.zeros((3, 96, 128), np.float32)
        for dy in range(3):
            out[dy] = wk[dy].reshape(96, 128)[:, perm]
        out[:, :, 0:96] *= 0.2
        return out

    in_maps = []
    for core in range(NCORES):
        b, half = core // 2, core % 2
        yb = y1[b] if half == 0 else y1[b, :, ::-1]
        yb = np.ascontiguousarray(yb[:, :RD])         # [T, 41, 64, 32]
        xim = np.zeros((97, T, RR, WW), np.float32)
        yt = yb.transpose(3, 0, 1, 2)                 # [32, T, 41, 64]
        for dx in range(3):
            xim[32 * dx:32 * dx + 32, :, 1:RR, 2 - dx:WW - dx] = yt
        xim[96] = 1.0
        ind = np.zeros((6, RR, WW), np.float32)
        ind[0] = 1.0            # ones / bias row
        ind[1, :, 1] = 1.0      # left col indicator (w=0)
        ind[2, :, 64] = 1.0     # right col (w=63)
        ind[3, 1, :] = 1.0      # top row (r=0)
        ind[4, 1, 1] = 1.0      # TL corner
        ind[5, 1, 64] = 1.0     # TR corner
        flip = half == 1
        in_maps.append({
            "xim": xim.astype(ml_dtypes.bfloat16),
            "w1": wmat1(kernels[0], biases[0], flip).astype(ml_dtypes.bfloat16),
            "u1": umat(rec_kernels[0], flip).astype(ml_dtypes.bfloat16),
            "w2": wmat2(kernels[1], biases[1], flip).astype(ml_dtypes.bfloat16),
            "u2": umat(rec_kernels[1], flip).astype(ml_dtypes.bfloat16),
            "ind": ind.astype(ml_dtypes.bfloat16),
        })
    return in_maps


def _decode(results):
    """Per-core out [T, 32, 2640] -> full [B,T,H,W,C]."""
    out = np.zeros((B, T, H, W, C), np.float32)
    for core in range(NCORES):
        o = results[core]["out"].astype(np.float32)   # [T, 32, 2640]
        o = o.reshape(T, 32, R, WW)[:, :, 0:32, 1:65]  # owned rows, valid w
        o = o.transpose(0, 2, 3, 1)                   # [T, 32, 64, 32]
        b, half = core // 2, core % 2
        if half == 0:
            out[b, :, 0:32] = o
        else:
            out[b, :, 32:64] = o[:, ::-1]
    return out


def kernel(**inputs):
    if "nc" not in _PROG:
        _PROG["nc"] = _build_program()
    in_maps = _host_prep(**inputs)
    res = run_bass_kernel_spmd(_PROG["nc"], in_maps, list(range(NCORES)))
    return _decode(res.results)
